# revision 11
# baseline (speedup 1.0000x reference)
"""RSNA loss kernel for Trainium2, SPMD across 8 NeuronCores.

Strategy (data-parallel over batch):
  - Shard B=128 exams -> 16 per core.
  - Per exam, view pred/label [8192, 10] as SBUF tile [128 part, 640]
    (partition p holds l in [64p, 64p+64), free index j*10+c, j=l%64).
  - The seq_len mask over (p, j) is rank-2:
        mask[p,j] = a[p]*s[j] + b[p]*t[j]
    with a=[p <= len//64], b=[p < len//64], s=[j < len%64], t=1-s.
    So masked channel sums become TWO TensorE matmuls per exam
    (contract partitions with lhsT columns a/b), followed by a
    j-weighted fold (multiply by host-built s/t patterns + reduce).
  - Image BCE: log(p0), log(1-p0) on ScalarE (strided channel-0 slice),
    bce = y0*(lp-lq)+lq on VectorE, masked-summed by the same a/b
    matmul + s/t fold trick.
  - Device outputs per core: [32, 21] partial sums; host does the tiny
    final combine (exam-level BCE on [128,9], scalar reduction) in f64.
All mask tensors are tiny host-built inputs derived from seq_lens.
"""
import numpy as np
from contextlib import ExitStack

import concourse.bass as bass
import concourse.bacc as bacc
import concourse.tile as tile
from concourse import mybir
from concourse.bass_utils import run_bass_kernel_spmd

N_CORES = 8
B, L, C = 128, 8192, 10
EPC = B // N_CORES          # exams per core = 16
JP = 64                     # l's per partition
NP = 128                    # partitions
COLS = JP * C               # 640 free columns per exam
BCEW = EPC * JP             # 1024 bce columns (16 exams x 64)

IMAGE_WEIGHT = 0.0736196319
EXAM_WEIGHTS = np.array([0.0736196319, 0.09202453988, 0.1042944785, 0.1042944785,
                         0.1877300613, 0.06257668712, 0.06257668712, 0.2346625767,
                         0.0782208589], dtype=np.float64)

_NC_CACHE = {}


def build_nc():
    nc = bacc.Bacc(trn_type="TRN2")
    f32 = mybir.dt.float32
    pred = nc.declare_dram_parameter("pred", [NP, EPC, COLS], f32, isOutput=False)
    label = nc.declare_dram_parameter("label", [NP, EPC, COLS], f32, isOutput=False)
    lhst = nc.declare_dram_parameter("lhst", [NP, EPC, 2 * EPC], f32, isOutput=False)
    lhstd = nc.declare_dram_parameter("lhstd", [NP, 2 * EPC], f32, isOutput=False)
    sstt = nc.declare_dram_parameter("sstt", [2 * EPC, COLS], f32, isOutput=False)
    ssttb = nc.declare_dram_parameter("ssttb", [2 * EPC, BCEW], f32, isOutput=False)
    out = nc.declare_dram_parameter("out", [2 * EPC, 21], f32, isOutput=True)

    with tile.TileContext(nc) as tc, ExitStack() as ctx:
        consts = ctx.enter_context(tc.tile_pool(name="consts", bufs=1))
        data = ctx.enter_context(tc.tile_pool(name="data", bufs=3))
        bcep = ctx.enter_context(tc.tile_pool(name="bcep", bufs=1))
        psum = ctx.enter_context(tc.tile_pool(name="psum", bufs=1, space="PSUM"))
        post = ctx.enter_context(tc.tile_pool(name="post", bufs=1))

        # constants
        t_lhst = consts.tile([NP, EPC, 2 * EPC], f32, tag="lhst")
        nc.sync.dma_start(out=t_lhst, in_=lhst[:, :, :])
        t_lhstd = consts.tile([NP, 2 * EPC], f32, tag="lhstd")
        nc.sync.dma_start(out=t_lhstd, in_=lhstd[:, :])
        t_sstt = consts.tile([2 * EPC, COLS], f32, tag="sstt")
        nc.sync.dma_start(out=t_sstt, in_=sstt[:, :])
        t_ssttb = consts.tile([2 * EPC, BCEW], f32, tag="ssttb")
        nc.sync.dma_start(out=t_ssttb, in_=ssttb[:, :])

        # batched channel-0 tiles
        LP = bcep.tile([NP, BCEW], f32, tag="LP")
        LQ = bcep.tile([NP, BCEW], f32, tag="LQ")
        Y0 = bcep.tile([NP, BCEW], f32, tag="Y0")
        T1 = bcep.tile([NP, BCEW], f32, tag="T1")
        BCE = bcep.tile([NP, BCEW], f32, tag="BCE")

        # psum accumulators
        P0p = psum.tile([2 * EPC, 320], f32, tag="P0p")
        P1p = psum.tile([2 * EPC, 320], f32, tag="P1p")
        P0l = psum.tile([2 * EPC, 320], f32, tag="P0l")
        P1l = psum.tile([2 * EPC, 320], f32, tag="P1l")
        PB0 = psum.tile([2 * EPC, 512], f32, tag="PB0")
        PB1 = psum.tile([2 * EPC, 512], f32, tag="PB1")

        GRP = 4                      # exams per DMA
        for tI in range(EPC // GRP):
            # one DMA per tensor loads GRP exams with 10KB-contiguous
            # per-partition runs; pred on the SP HWDGE ring, label on ACT's
            Tp = data.tile([NP, GRP, COLS], f32, tag="Tp")
            nc.sync.dma_start(
                out=Tp, in_=pred[:, GRP * tI:GRP * (tI + 1), :])
            Tl = data.tile([NP, GRP, COLS], f32, tag="Tl")
            nc.scalar.dma_start(
                out=Tl, in_=label[:, GRP * tI:GRP * (tI + 1), :])
            for eo in range(GRP):
                e = GRP * tI + eo
                lhsT_e = t_lhst[:, e, :]
                st = dict(start=(e == 0), stop=(e == EPC - 1))
                nc.tensor.matmul(P0p, lhsT_e, Tp[:, eo, 0:320], **st)
                nc.tensor.matmul(P1p, lhsT_e, Tp[:, eo, 320:640], **st)
                nc.tensor.matmul(P0l, lhsT_e, Tl[:, eo, 0:320], **st)
                nc.tensor.matmul(P1l, lhsT_e, Tl[:, eo, 320:640], **st)
                # channel-0 strided views [128, 64]
                p0 = Tp[:, eo].rearrange("p (j c) -> p j c", c=C)[:, :, 0]
                y0 = Tl[:, eo].rearrange("p (j c) -> p j c", c=C)[:, :, 0]
                sl = slice(JP * e, JP * (e + 1))
                nc.scalar.activation(
                    out=LP[:, sl], in_=p0,
                    func=mybir.ActivationFunctionType.Ln)
                nc.scalar.activation(
                    out=LQ[:, sl], in_=p0,
                    func=mybir.ActivationFunctionType.Ln, bias=1.0, scale=-1.0)
                nc.vector.tensor_copy(Y0[:, sl], y0)

        # bce = y0*(lp - lq) + lq
        nc.vector.tensor_sub(T1, LP, LQ)
        nc.vector.tensor_mul(T1, T1, Y0)
        nc.vector.tensor_add(BCE, T1, LQ)
        nc.tensor.matmul(PB0, t_lhstd, BCE[:, 0:512], start=True, stop=True)
        nc.tensor.matmul(PB1, t_lhstd, BCE[:, 512:1024], start=True, stop=True)

        # post: weight by s/t patterns, fold j, emit [32, 21]
        O = post.tile([2 * EPC, 21], f32, tag="O")
        W0 = post.tile([2 * EPC, 320], f32, tag="W0")
        W1 = post.tile([2 * EPC, 320], f32, tag="W1")
        for (Pa, Pb, oc) in ((P0p, P1p, 0), (P0l, P1l, 10)):
            nc.vector.tensor_mul(W0, Pa, t_sstt[:, 0:320])
            nc.vector.tensor_mul(W1, Pb, t_sstt[:, 320:640])
            nc.vector.tensor_add(W0, W0, W1)
            nc.vector.tensor_reduce(
                out=O[:, oc:oc + 10],
                in_=W0.rearrange("p (j c) -> p c j", c=C),
                axis=mybir.AxisListType.X, op=mybir.AluOpType.add)
        WB0 = post.tile([2 * EPC, 512], f32, tag="WB0")
        WB1 = post.tile([2 * EPC, 512], f32, tag="WB1")
        nc.vector.tensor_mul(WB0, PB0, t_ssttb[:, 0:512])
        nc.vector.tensor_mul(WB1, PB1, t_ssttb[:, 512:1024])
        nc.vector.tensor_add(WB0, WB0, WB1)
        nc.vector.tensor_reduce(
            out=O[:, 20:21], in_=WB0,
            axis=mybir.AxisListType.X, op=mybir.AluOpType.add)
        nc.sync.dma_start(out=out[:, :], in_=O)
    nc.finalize()
    return nc


def _mask_tensors(lens):
    """Per-core mask inputs from the 16 seq_lens of this core."""
    lhst = np.zeros((EPC, NP, 2 * EPC), np.float32)
    lhstd = np.zeros((NP, 2 * EPC), np.float32)
    sstt = np.zeros((2 * EPC, COLS), np.float32)
    ssttb = np.zeros((2 * EPC, BCEW), np.float32)
    p_idx = np.arange(NP)
    j_idx = np.arange(JP)
    for e, ln in enumerate(lens):
        P, r = divmod(int(ln), JP)
        a = (p_idx <= P).astype(np.float32)
        b = (p_idx < P).astype(np.float32)
        s = (j_idx < r).astype(np.float32)
        t = 1.0 - s
        lhst[e, :, 2 * e] = a
        lhst[e, :, 2 * e + 1] = b
        lhstd[:, 2 * e] = a
        lhstd[:, 2 * e + 1] = b
        sstt[2 * e, :] = np.repeat(s, C)
        sstt[2 * e + 1, :] = np.repeat(t, C)
        ssttb[2 * e, JP * e:JP * (e + 1)] = s
        ssttb[2 * e + 1, JP * e:JP * (e + 1)] = t
    return np.ascontiguousarray(lhst.transpose(1, 0, 2)), lhstd, sstt, ssttb


def make_in_maps(pred, label, seq_lens):
    in_maps = []
    for i in range(N_CORES):
        sl = slice(i * EPC, (i + 1) * EPC)
        lhst, lhstd, sstt, ssttb = _mask_tensors(seq_lens[sl])
        in_maps.append({
            "pred": np.ascontiguousarray(
                pred[sl].reshape(EPC, NP, COLS).transpose(1, 0, 2)),
            "label": np.ascontiguousarray(
                label[sl].reshape(EPC, NP, COLS).transpose(1, 0, 2)),
            "lhst": lhst, "lhstd": lhstd, "sstt": sstt, "ssttb": ssttb,
        })
    return in_maps


def finish(outs, seq_lens):
    """Host-side final combine from the 8 per-core [32, 21] outputs."""
    w = EXAM_WEIGHTS
    exam_loss = 0.0
    image_loss = 0.0
    tw_img = 0.0
    for i in range(N_CORES):
        O = outs[i].astype(np.float64)
        S = O[0::2] + O[1::2]                 # [16, 21] a+b parts
        lens = seq_lens[i * EPC:(i + 1) * EPC].astype(np.float64)
        pm = S[:, 1:10] / lens[:, None]
        ym = S[:, 11:20] / lens[:, None]
        exam_bce = -(ym * np.log(pm) + (1.0 - ym) * np.log(1.0 - pm))
        exam_loss += float(np.sum(exam_bce * w[None, :]))
        y0m = S[:, 10] / lens
        imgw = IMAGE_WEIGHT * y0m
        image_loss += float(np.sum(-S[:, 20] * imgw))
        tw_img += float(np.sum(imgw * lens))
    total_weights = B * float(np.sum(w)) + tw_img
    return np.float32((exam_loss + image_loss) / total_weights)


def kernel(pred, label, seq_lens):
    if "nc" not in _NC_CACHE:
        _NC_CACHE["nc"] = build_nc()
    nc = _NC_CACHE["nc"]
    in_maps = make_in_maps(np.asarray(pred), np.asarray(label),
                           np.asarray(seq_lens))
    res = run_bass_kernel_spmd(nc, in_maps, core_ids=list(range(N_CORES)))
    outs = [res.results[i]["out"] for i in range(N_CORES)]
    return finish(outs, np.asarray(seq_lens))


if __name__ == "__main__":
    rng = np.random.default_rng(0)
    pred = (rng.random((B, L, C), np.float32) * 0.98 + 0.01).astype(np.float32)
    label = (rng.random((B, L, C), np.float32) * 0.98 + 0.01).astype(np.float32)
    seq_lens = rng.integers(1, L + 1, size=(B,)).astype(np.int32)
    got = kernel(pred=pred, label=label, seq_lens=seq_lens)
    print("kernel:", got)


# revision 16
# speedup vs baseline: 1.1050x; 1.1050x over previous
"""RSNA loss kernel for Trainium2, SPMD across 8 NeuronCores.

Strategy (data-parallel over batch):
  - Shard B=128 exams -> 16 per core.
  - Per exam, view pred/label [8192, 10] as SBUF tile [128 part, 640]
    (partition p holds l in [64p, 64p+64), free index j*10+c, j=l%64).
  - The seq_len mask over (p, j) is rank-2:
        mask[p,j] = a[p]*s[j] + b[p]*t[j]
    with a=[p <= len//64], b=[p < len//64], s=[j < len%64], t=1-s.
    So masked channel sums become TWO TensorE matmuls per exam
    (contract partitions with lhsT columns a/b), followed by a
    j-weighted fold (multiply by host-built s/t patterns + reduce).
  - Image BCE: log(p0), log(1-p0) on ScalarE (strided channel-0 slice),
    bce = y0*(lp-lq)+lq on VectorE, masked-summed by the same a/b
    matmul + s/t fold trick.
  - Device outputs per core: [32, 21] partial sums; host does the tiny
    final combine (exam-level BCE on [128,9], scalar reduction) in f64.
All mask tensors are tiny host-built inputs derived from seq_lens.
"""
import numpy as np
from contextlib import ExitStack

import concourse.bass as bass
import concourse.bacc as bacc
import concourse.tile as tile
from concourse import mybir
from concourse.bass_utils import run_bass_kernel_spmd

N_CORES = 8
B, L, C = 128, 8192, 10
EPC = B // N_CORES          # exams per core = 16
JP = 64                     # l's per partition
NP = 128                    # partitions
COLS = JP * C               # 640 free columns per exam
BCEW = EPC * JP             # 1024 bce columns (16 exams x 64)

IMAGE_WEIGHT = 0.0736196319
EXAM_WEIGHTS = np.array([0.0736196319, 0.09202453988, 0.1042944785, 0.1042944785,
                         0.1877300613, 0.06257668712, 0.06257668712, 0.2346625767,
                         0.0782208589], dtype=np.float64)

_NC_CACHE = {}


def build_nc():
    nc = bacc.Bacc(trn_type="TRN2")
    f32 = mybir.dt.float32
    pred = nc.declare_dram_parameter("pred", [NP, EPC, COLS], f32, isOutput=False)
    label = nc.declare_dram_parameter("label", [NP, EPC, COLS], f32, isOutput=False)
    bf16 = mybir.dt.bfloat16
    lhst = nc.declare_dram_parameter("lhst", [NP, EPC, 2 * EPC], bf16, isOutput=False)
    lhstd = nc.declare_dram_parameter("lhstd", [NP, 2 * EPC], f32, isOutput=False)
    sstt = nc.declare_dram_parameter("sstt", [2 * EPC, COLS], f32, isOutput=False)
    ssttb = nc.declare_dram_parameter("ssttb", [2 * EPC, BCEW], f32, isOutput=False)
    out = nc.declare_dram_parameter("out", [2 * EPC, 21], f32, isOutput=True)

    with tile.TileContext(nc) as tc, ExitStack() as ctx:
        consts = ctx.enter_context(tc.tile_pool(name="consts", bufs=1))
        data = ctx.enter_context(tc.tile_pool(name="data", bufs=4))
        conv = ctx.enter_context(tc.tile_pool(name="conv", bufs=2))
        bcep = ctx.enter_context(tc.tile_pool(name="bcep", bufs=1))
        psum = ctx.enter_context(tc.tile_pool(name="psum", bufs=1, space="PSUM"))
        post = ctx.enter_context(tc.tile_pool(name="post", bufs=1))

        # constants
        t_lhst = consts.tile([NP, EPC, 2 * EPC], bf16, tag="lhst")
        nc.sync.dma_start(out=t_lhst, in_=lhst[:, :, :])
        t_lhstd = consts.tile([NP, 2 * EPC], f32, tag="lhstd")
        nc.sync.dma_start(out=t_lhstd, in_=lhstd[:, :])
        t_sstt = consts.tile([2 * EPC, COLS], f32, tag="sstt")
        nc.sync.dma_start(out=t_sstt, in_=sstt[:, :])
        t_ssttb = consts.tile([2 * EPC, BCEW], f32, tag="ssttb")
        nc.sync.dma_start(out=t_ssttb, in_=ssttb[:, :])

        # batched channel-0 tiles
        LP = bcep.tile([NP, BCEW], f32, tag="LP")
        LQ = bcep.tile([NP, BCEW], f32, tag="LQ")
        Y0 = bcep.tile([NP, BCEW], f32, tag="Y0")
        T1 = bcep.tile([NP, BCEW], f32, tag="T1")
        BCE = bcep.tile([NP, BCEW], f32, tag="BCE")

        # psum accumulators
        P0p = psum.tile([2 * EPC, 320], f32, tag="P0p")
        P1p = psum.tile([2 * EPC, 320], f32, tag="P1p")
        P0l = psum.tile([2 * EPC, 320], f32, tag="P0l")
        P1l = psum.tile([2 * EPC, 320], f32, tag="P1l")
        PB0 = psum.tile([2 * EPC, 512], f32, tag="PB0")
        PB1 = psum.tile([2 * EPC, 512], f32, tag="PB1")

        GRP = 4                      # exams per DMA
        for tI in range(EPC // GRP):
            # one DMA per tensor loads GRP exams with 10KB-contiguous
            # per-partition runs; pred on the SP HWDGE ring, label on ACT's
            Tp = data.tile([NP, GRP, COLS], f32, tag="Tp")
            nc.sync.dma_start(
                out=Tp, in_=pred[:, GRP * tI:GRP * (tI + 1), :])
            Tl = data.tile([NP, GRP, COLS], f32, tag="Tl")
            nc.gpsimd.dma_start(
                out=Tl, in_=label[:, GRP * tI:GRP * (tI + 1), :])
            # bf16 copies for the TensorE path (2x matmul throughput);
            # pred on DVE, label on ACT — both have slack
            TpB = conv.tile([NP, GRP, COLS], bf16, tag="TpB")
            nc.vector.tensor_copy(TpB, Tp)
            TlB = conv.tile([NP, GRP, COLS], bf16, tag="TlB")
            nc.scalar.copy(TlB, Tl)
            for eo in range(GRP):
                e = GRP * tI + eo
                lhsT_e = t_lhst[:, e, :]
                st = dict(start=(e == 0), stop=(e == EPC - 1))
                nc.tensor.matmul(P0p, lhsT_e, TpB[:, eo, 0:320], **st)
                nc.tensor.matmul(P1p, lhsT_e, TpB[:, eo, 320:640], **st)
                nc.tensor.matmul(P0l, lhsT_e, TlB[:, eo, 0:320], **st)
                nc.tensor.matmul(P1l, lhsT_e, TlB[:, eo, 320:640], **st)
                # channel-0 strided views [128, 64]
                p0 = Tp[:, eo].rearrange("p (j c) -> p j c", c=C)[:, :, 0]
                y0 = Tl[:, eo].rearrange("p (j c) -> p j c", c=C)[:, :, 0]
                sl = slice(JP * e, JP * (e + 1))
                nc.scalar.activation(
                    out=LP[:, sl], in_=p0,
                    func=mybir.ActivationFunctionType.Ln)
                nc.scalar.activation(
                    out=LQ[:, sl], in_=p0,
                    func=mybir.ActivationFunctionType.Ln, bias=1.0, scale=-1.0)
                nc.vector.tensor_copy(Y0[:, sl], y0)

        # bce = y0*(lp - lq) + lq
        nc.vector.tensor_sub(T1, LP, LQ)
        nc.vector.tensor_mul(T1, T1, Y0)
        nc.vector.tensor_add(BCE, T1, LQ)
        nc.tensor.matmul(PB0, t_lhstd, BCE[:, 0:512], start=True, stop=True)
        nc.tensor.matmul(PB1, t_lhstd, BCE[:, 512:1024], start=True, stop=True)

        # post: weight by s/t patterns, fold j, emit [32, 21]
        O = post.tile([2 * EPC, 21], f32, tag="O")
        W0 = post.tile([2 * EPC, 320], f32, tag="W0")
        W1 = post.tile([2 * EPC, 320], f32, tag="W1")
        for (Pa, Pb, oc) in ((P0p, P1p, 0), (P0l, P1l, 10)):
            nc.vector.tensor_mul(W0, Pa, t_sstt[:, 0:320])
            nc.vector.tensor_mul(W1, Pb, t_sstt[:, 320:640])
            nc.vector.tensor_add(W0, W0, W1)
            nc.vector.tensor_reduce(
                out=O[:, oc:oc + 10],
                in_=W0.rearrange("p (j c) -> p c j", c=C),
                axis=mybir.AxisListType.X, op=mybir.AluOpType.add)
        WB0 = post.tile([2 * EPC, 512], f32, tag="WB0")
        WB1 = post.tile([2 * EPC, 512], f32, tag="WB1")
        nc.vector.tensor_mul(WB0, PB0, t_ssttb[:, 0:512])
        nc.vector.tensor_mul(WB1, PB1, t_ssttb[:, 512:1024])
        nc.vector.tensor_add(WB0, WB0, WB1)
        nc.vector.tensor_reduce(
            out=O[:, 20:21], in_=WB0,
            axis=mybir.AxisListType.X, op=mybir.AluOpType.add)
        nc.sync.dma_start(out=out[:, :], in_=O)
    nc.finalize()
    return nc


def _mask_tensors(lens):
    """Per-core mask inputs from the 16 seq_lens of this core."""
    lhst = np.zeros((EPC, NP, 2 * EPC), np.float32)
    lhstd = np.zeros((NP, 2 * EPC), np.float32)
    sstt = np.zeros((2 * EPC, COLS), np.float32)
    ssttb = np.zeros((2 * EPC, BCEW), np.float32)
    p_idx = np.arange(NP)
    j_idx = np.arange(JP)
    for e, ln in enumerate(lens):
        P, r = divmod(int(ln), JP)
        a = (p_idx <= P).astype(np.float32)
        b = (p_idx < P).astype(np.float32)
        s = (j_idx < r).astype(np.float32)
        t = 1.0 - s
        lhst[e, :, 2 * e] = a
        lhst[e, :, 2 * e + 1] = b
        lhstd[:, 2 * e] = a
        lhstd[:, 2 * e + 1] = b
        sstt[2 * e, :] = np.repeat(s, C)
        sstt[2 * e + 1, :] = np.repeat(t, C)
        ssttb[2 * e, JP * e:JP * (e + 1)] = s
        ssttb[2 * e + 1, JP * e:JP * (e + 1)] = t
    bf16np = mybir.dt.np(mybir.dt.bfloat16)
    return (np.ascontiguousarray(lhst.transpose(1, 0, 2)).astype(bf16np),
            lhstd, sstt, ssttb)


def make_in_maps(pred, label, seq_lens):
    in_maps = []
    for i in range(N_CORES):
        sl = slice(i * EPC, (i + 1) * EPC)
        lhst, lhstd, sstt, ssttb = _mask_tensors(seq_lens[sl])
        in_maps.append({
            "pred": np.ascontiguousarray(
                pred[sl].reshape(EPC, NP, COLS).transpose(1, 0, 2)),
            "label": np.ascontiguousarray(
                label[sl].reshape(EPC, NP, COLS).transpose(1, 0, 2)),
            "lhst": lhst, "lhstd": lhstd, "sstt": sstt, "ssttb": ssttb,
        })
    return in_maps


def finish(outs, seq_lens):
    """Host-side final combine from the 8 per-core [32, 21] outputs."""
    w = EXAM_WEIGHTS
    exam_loss = 0.0
    image_loss = 0.0
    tw_img = 0.0
    for i in range(N_CORES):
        O = outs[i].astype(np.float64)
        S = O[0::2] + O[1::2]                 # [16, 21] a+b parts
        lens = seq_lens[i * EPC:(i + 1) * EPC].astype(np.float64)
        pm = S[:, 1:10] / lens[:, None]
        ym = S[:, 11:20] / lens[:, None]
        exam_bce = -(ym * np.log(pm) + (1.0 - ym) * np.log(1.0 - pm))
        exam_loss += float(np.sum(exam_bce * w[None, :]))
        y0m = S[:, 10] / lens
        imgw = IMAGE_WEIGHT * y0m
        image_loss += float(np.sum(-S[:, 20] * imgw))
        tw_img += float(np.sum(imgw * lens))
    total_weights = B * float(np.sum(w)) + tw_img
    return np.float32((exam_loss + image_loss) / total_weights)


def kernel(pred, label, seq_lens):
    if "nc" not in _NC_CACHE:
        _NC_CACHE["nc"] = build_nc()
    nc = _NC_CACHE["nc"]
    in_maps = make_in_maps(np.asarray(pred), np.asarray(label),
                           np.asarray(seq_lens))
    res = run_bass_kernel_spmd(nc, in_maps, core_ids=list(range(N_CORES)))
    outs = [res.results[i]["out"] for i in range(N_CORES)]
    return finish(outs, np.asarray(seq_lens))


if __name__ == "__main__":
    rng = np.random.default_rng(0)
    pred = (rng.random((B, L, C), np.float32) * 0.98 + 0.01).astype(np.float32)
    label = (rng.random((B, L, C), np.float32) * 0.98 + 0.01).astype(np.float32)
    seq_lens = rng.integers(1, L + 1, size=(B,)).astype(np.int32)
    got = kernel(pred=pred, label=label, seq_lens=seq_lens)
    print("kernel:", got)
